# revision 1
# baseline (speedup 1.0000x reference)
"""Linear attention (elu+1 feature map) Bass/Tile kernel for Trainium2.

Full inputs: queries/keys/values [N=8, L/S=8192, H=8, D=64] fp32.
Sharding: data-parallel over N across the 8 NeuronCores (batch i -> core i).

Math per (n, h):
  Q' = elu(Q)+1, K' = elu(K)+1
  KV[d, v] = sum_s K'[s, d] V[s, v]     (the /S, *S in the reference cancel
  Ksum[d]  = sum_s K'[s, d]              exactly in fp32: S = 2^13)
  out[l, v] = (Q'[l, :] @ KV[:, v]) / (Q'[l, :] @ Ksum + eps)

Kernel structure per core:
  Phase 1 (stream K, V):  per 128-row chunk, feature-map K on ACT+DVE, then
    per head one matmul  lhsT=K'_h [128, 64], rhs=[V_h | ones] [128, 65]
    accumulated into PSUM [KV | Ksum].  Head pairs share a PSUM bank via
    tile_position col tiling ((0,0) / (0,64)).
  Phase 2 (stream Q): per 128-row chunk, PE-transpose raw Q ([128 l, 128 2d]
    -> [128 2d, 128 l]), apply elu+1 during the PSUM drain (ACT relu(-x),
    ACT exp(-t), DVE (max(x,0)+e)), then block-diag matmul
    lhsT=Q'^T-pair [128, 128], rhs=W2aug [128, 130] -> psum [128 l, 65+65]
    with out columns and the denominator column per head; epilogue divides
    on DVE and DMAs out in natural [l, (h v)] layout.
"""

import functools
import sys

sys.path.insert(0, "/opt/trn_rl_repo")

import numpy as np

import concourse.bass as bass
import concourse.mybir as mybir
import concourse.tile as tile
from concourse import bacc
from concourse.bass_utils import run_bass_kernel_spmd
from concourse.masks import make_identity

N, L, S, H, D = 8, 8192, 8192, 8, 64
EPS = 1e-6
P = 128
FP32 = mybir.dt.float32
AF = mybir.ActivationFunctionType
OP = mybir.AluOpType


def _feature_map(nc, pools, x_ap, out_ap, shape, tag, split=False):
    """out = elu(x)+1 = max(x,0) + exp(min(x,0)).

    Fused form (split=False): ACT t = relu(-x); ACT e = exp(-t);
    DVE out = (x max 0) + e.  Used when x comes from PSUM (PE) so the DVE
    op sees only 2 distinct upstream semaphores (PE + ACT).

    Split form (split=True): same t, e; then DVE s = t + e;
    DVE out = x + s  (relu(x) = x + relu(-x), so x + t + e = elu(x)+1).
    Keeps every instruction at <=2 distinct semaphore waits when x comes
    from a DMA (walrus rejects >2 sync waits per ACT/STT instruction).
    """
    t = pools.tile(shape, FP32, name=f"fm_t_{tag}", tag=f"fm_t_{tag}")
    e = pools.tile(shape, FP32, name=f"fm_e_{tag}", tag=f"fm_e_{tag}")
    nc.scalar.activation(t, x_ap, AF.Relu, scale=-1.0)
    nc.scalar.activation(e, t, AF.Exp, scale=-1.0)
    if split:
        s = pools.tile(shape, FP32, name=f"fm_s_{tag}", tag=f"fm_s_{tag}")
        nc.vector.tensor_add(s, t, e)
        nc.vector.tensor_add(out_ap, x_ap, s)
    else:
        nc.vector.scalar_tensor_tensor(
            out_ap, in0=x_ap, scalar=0.0, in1=e, op0=OP.max, op1=OP.add
        )


def build_kernel(L_=L, S_=S):
    nc = bacc.Bacc(trn_type="TRN2")
    HD = H * D
    q_d = nc.dram_tensor("queries", [L_, HD], FP32, kind="ExternalInput")
    k_d = nc.dram_tensor("keys", [S_, HD], FP32, kind="ExternalInput")
    v_d = nc.dram_tensor("values", [S_, HD], FP32, kind="ExternalInput")
    o_d = nc.dram_tensor("out", [L_, HD], FP32, kind="ExternalOutput")

    n_kc = S_ // 256  # K/V outer iterations (2 chunks of 128 each)
    n_qc = L_ // 256

    with tile.TileContext(nc) as tc:
        with (
            tc.tile_pool(name="consts", bufs=1) as consts,
            tc.tile_pool(name="kdma", bufs=3) as kdma,
            tc.tile_pool(name="vdma", bufs=3) as vdma,
            tc.tile_pool(name="fmk", bufs=2) as fmk,
            tc.tile_pool(name="w2p", bufs=1) as w2p,
            tc.tile_pool(name="qdma", bufs=3) as qdma,
            tc.tile_pool(name="kvpsum", bufs=1, space="PSUM") as kvpsum,
            tc.tile_pool(name="pst", bufs=2, space="PSUM") as pstp,
            tc.tile_pool(name="psum2", bufs=1, space="PSUM") as psum2p,
            tc.tile_pool(name="fmq", bufs=2) as fmq,
            tc.tile_pool(name="qt", bufs=2) as qtp,
            tc.tile_pool(name="zp", bufs=2) as zp,
            tc.tile_pool(name="outp", bufs=3) as outp,
        ):
            ident = consts.tile([P, P], FP32)
            make_identity(nc, ident)

            # ---- Phase 1: KV + Ksum accumulation ----
            # 4 psum tiles, one bank per head PAIR.  One matmul per pair:
            # lhsT = K'[128 s, 128 (2 heads d)], rhs = [V_pair | ones]
            # [128, 129] -> psum [128, 129]: KV_2j at [0:64, 0:64],
            # KV_2j+1 at [64:128, 64:128], Ksums in col 128 (cross blocks
            # are unused garbage).
            kv_ps = [kvpsum.tile([P, 129], FP32, name=f"kv{j}", tag=f"kv{j}") for j in range(4)]

            for cc in range(n_kc):
                r0 = cc * 256
                ktile = kdma.tile([P, 2, HD], FP32, name="ktile", tag="ktile")
                nc.sync.dma_start(
                    ktile,
                    k_d[r0 : r0 + 256, :].rearrange("(two p) f -> p two f", p=P),
                )
                vtile = vdma.tile([P, 2, 4, 129], FP32, name="vtile", tag="vtile")
                nc.vector.memset(vtile[:, :, :, 128:129], 1.0)
                for sub in range(2):
                    nc.sync.dma_start(
                        vtile[:, sub, :, 0:128],
                        v_d[r0 + sub * P : r0 + (sub + 1) * P, :].rearrange(
                            "p (j e) -> p j e", j=4
                        ),
                    )
                kp = fmk.tile([P, 2, H, D], FP32, name="kp", tag="kp")
                _feature_map(
                    nc, fmk, ktile.rearrange("p two (h d) -> p two h d", h=H), kp,
                    [P, 2, H, D], "k", split=True,
                )
                kpf = kp.rearrange("p two h d -> p two (h d)")
                for sub in range(2):
                    for j in range(4):
                        nc.tensor.matmul(
                            kv_ps[j],
                            lhsT=kpf[:, sub, j * P : (j + 1) * P],
                            rhs=vtile[:, sub, j, :],
                            start=(cc == 0 and sub == 0),
                            stop=(cc == n_kc - 1 and sub == 1),
                        )

            # ---- Phase 1.5: build block-diagonal [KV | Ksum] weights ----
            # w2[j] [128, 130]: cols 0:65 = head 2j rows 0:64; cols 65:130 =
            # head 2j+1 rows 64:128; rest zero.
            w2 = [w2p.tile([P, 130], FP32, name=f"w2_{j}", tag=f"w2_{j}") for j in range(4)]
            for j in range(4):
                nc.vector.memset(w2[j], 0.0)
                nc.vector.tensor_copy(w2[j][0:64, 0:64], kv_ps[j][0:64, 0:64])
                nc.vector.tensor_copy(w2[j][0:64, 64:65], kv_ps[j][0:64, 128:129])
                nc.vector.tensor_copy(w2[j][64:128, 65:129], kv_ps[j][64:128, 64:128])
                nc.vector.tensor_copy(w2[j][64:128, 129:130], kv_ps[j][64:128, 128:129])

            # ---- Phase 2: stream Q ----
            for cc in range(n_qc):
                r0 = cc * 256
                qtile = qdma.tile([P, 2, HD], FP32, name="qtile", tag="qtile")
                nc.sync.dma_start(
                    qtile,
                    q_d[r0 : r0 + 256, :].rearrange("(two p) f -> p two f", p=P),
                )
                for sub in range(2):
                    # PE transpose raw Q: [128 l, 128 (2 heads d)] -> [128, 128 l]
                    pst = pstp.tile([P, HD], FP32, name="pst", tag="pst")
                    for g in range(4):
                        nc.tensor.transpose(
                            pst[:, g * P : (g + 1) * P],
                            qtile[:, sub, g * P : (g + 1) * P],
                            ident,
                        )
                    qt = qtp.tile([P, HD], FP32, name="qt", tag="qt")
                    _feature_map(nc, fmq, pst, qt, [P, HD], "q")

                    otile = outp.tile([P, H, D], FP32, name="otile", tag="otile")
                    for g2 in range(2):
                        p2 = psum2p.tile([P, 260], FP32, name=f"p2_{g2}", tag=f"p2_{g2}")
                        for gg in range(2):
                            g = 2 * g2 + gg
                            nc.tensor.matmul(
                                p2[:, gg * 130 : (gg + 1) * 130],
                                lhsT=qt[:, g * P : (g + 1) * P],
                                rhs=w2[g],
                                start=True,
                                stop=True,
                            )
                        p2r = p2.rearrange("p (b c) -> p b c", c=65)
                        zt = zp.tile([P, 4], FP32, name=f"zt{g2}", tag=f"zt{g2}")
                        nc.vector.tensor_scalar_add(zt, p2r[:, :, 64], EPS)
                        zr = zp.tile([P, 4], FP32, name=f"zr{g2}", tag=f"zr{g2}")
                        nc.vector.reciprocal(zr, zt)
                        for b in range(4):
                            nc.vector.tensor_scalar_mul(
                                otile[:, 4 * g2 + b, :],
                                p2r[:, b, 0:64],
                                zr[:, b : b + 1],
                            )
                    nc.sync.dma_start(
                        o_d[r0 + sub * P : r0 + (sub + 1) * P, :],
                        otile.rearrange("p h d -> p (h d)"),
                    )
    nc.compile()
    return nc


@functools.lru_cache(maxsize=None)
def _cached_nc(L_, S_):
    return build_kernel(L_, S_)


def kernel(queries: np.ndarray, keys: np.ndarray, values: np.ndarray) -> np.ndarray:
    n, l_, h, d = queries.shape
    s_ = keys.shape[1]
    nc = _cached_nc(l_, s_)
    in_maps = [
        {
            "queries": np.ascontiguousarray(queries[i].reshape(l_, h * d), np.float32),
            "keys": np.ascontiguousarray(keys[i].reshape(s_, h * d), np.float32),
            "values": np.ascontiguousarray(values[i].reshape(s_, h * d), np.float32),
        }
        for i in range(n)
    ]
    res = run_bass_kernel_spmd(nc, in_maps, core_ids=list(range(n)))
    return np.stack(
        [res.results[i]["out"].reshape(l_, h, d) for i in range(n)]
    ).astype(np.float32)


if __name__ == "__main__":
    # smoke build
    nc = build_kernel()
    print("build ok")



# revision 3
# speedup vs baseline: 434.4556x; 434.4556x over previous
"""Linear attention (elu+1 feature map) Bass/Tile kernel for Trainium2.

Full inputs: queries/keys/values [N=8, L/S=8192, H=8, D=64] fp32.
Sharding: data-parallel over N across the 8 NeuronCores (batch i -> core i).
Inputs are cast to bf16 on the host (round-to-nearest) to halve DMA volume
and enable bf16 matmul / FWL / fast DVE modes; all PSUM accumulation is fp32.

Math per (n, h):
  Q' = elu(Q)+1, K' = elu(K)+1
  KV[d, v] = sum_s K'[s, d] V[s, v]     (the /S, *S in the reference cancel
  Ksum[d]  = sum_s K'[s, d]              exactly: S = 2^13)
  out[l, v] = (Q'[l, :] @ KV[:, v]) / (Q'[l, :] @ Ksum)   (EPS=1e-6 dropped:
  denom ~ O(5000), the reference's eps is far below bf16 noise)

Kernel structure per core:
  Phase 1 (stream K, V in 512-row chunks): feature-map K on ACT+DVE, then per
    head PAIR one matmul lhsT=K'_pair [128, 128], rhs=V_pair [128, 128]
    accumulated into psum KV_j [128, 128] (cross-head blocks are computed but
    discarded), plus a tiny N=1 matmul with a ones column accumulating
    Ksum into a shared [128, 4] psum bank.
  Phase 1.5: copy KV block-diagonals to bf16 weights w2[j] [128, 128]
    (cross blocks zeroed) and scatter Ksum into block rhs kb[g] [128, 8].
  Phase 2 (stream Q in 2048-row mega-chunks): per head-group g, xbar
    DMA-transpose loads Q^T [128 hd, 2048 l] directly from DRAM; feature-map;
    then per 128-l sub-chunk: 4 matmuls (lhsT=Q'^T slice, rhs=w2[g]) fill one
    [128, 512] out psum + 4 N=8 matmuls accumulate all-head denominators into
    a [128, 32] psum (4 subs batched); one reciprocal per 4 subs; epilogue is
    a single tensor_tensor multiply with a stride-0 broadcast of 1/denom.
"""

import functools
import sys

sys.path.insert(0, "/opt/trn_rl_repo")

import numpy as np
import ml_dtypes

import concourse.bass as bass
import concourse.mybir as mybir
import concourse.tile as tile
from concourse import bacc
from concourse.bass_utils import run_bass_kernel_spmd

N, L, S, H, D = 8, 8192, 8192, 8, 64
P = 128
HD = H * D
BF16 = mybir.dt.bfloat16
FP32 = mybir.dt.float32
AF = mybir.ActivationFunctionType
OP = mybir.AluOpType
KC = 512    # K/V chunk rows
QC = 2048   # Q mega-chunk rows


def build_kernel(L_=L, S_=S):
    nc = bacc.Bacc(trn_type="TRN2")
    q_d = nc.dram_tensor("queries", [L_, HD], BF16, kind="ExternalInput")
    k_d = nc.dram_tensor("keys", [S_, HD], BF16, kind="ExternalInput")
    v_d = nc.dram_tensor("values", [S_, HD], BF16, kind="ExternalInput")
    o_d = nc.dram_tensor("out", [L_, HD], FP32, kind="ExternalOutput")

    n_kc = S_ // KC
    n_qc = L_ // QC

    with tile.TileContext(nc) as tc:
        with (
            tc.tile_pool(name="consts", bufs=1) as consts,
            tc.tile_pool(name="kdma", bufs=3) as kdma,
            tc.tile_pool(name="vdma", bufs=3) as vdma,
            tc.tile_pool(name="fmk", bufs=2) as fmk,
            tc.tile_pool(name="wp", bufs=1) as wp,
            tc.tile_pool(name="qdma", bufs=2) as qdma,
            tc.tile_pool(name="fmq", bufs=2) as fmq,
            tc.tile_pool(name="zp", bufs=2) as zp,
            tc.tile_pool(name="outp", bufs=3) as outp,
            tc.tile_pool(name="kvps", bufs=1, space="PSUM") as kvps,
            tc.tile_pool(name="ksps", bufs=1, space="PSUM") as ksps,
            tc.tile_pool(name="ops", bufs=2, space="PSUM") as opsp,
            tc.tile_pool(name="dps", bufs=1, space="PSUM") as dpsp,
        ):
            ones = consts.tile([P, 1], BF16)
            nc.vector.memset(ones, 1.0)

            # ---- Phase 1: KV + Ksum accumulation ----
            kv_ps = [
                kvps.tile([P, P], FP32, name=f"kv{j}", tag=f"kv{j}") for j in range(4)
            ]
            ks_ps = ksps.tile([P, 4], FP32, name="ks", tag="ks")

            for cc in range(n_kc):
                r0 = cc * KC
                ktile = kdma.tile([P, 4, HD], BF16, name="ktile", tag="ktile")
                nc.sync.dma_start(
                    ktile, k_d[r0 : r0 + KC, :].rearrange("(c p) f -> p c f", p=P)
                )
                vtile = vdma.tile([P, 4, HD], BF16, name="vtile", tag="vtile")
                nc.sync.dma_start(
                    vtile, v_d[r0 : r0 + KC, :].rearrange("(c p) f -> p c f", p=P)
                )
                # kp = elu(ktile)+1 = max(x,0) + exp(-relu(-x))
                t = fmk.tile([P, 4, HD], BF16, name="fk_t", tag="fk_t")
                e = fmk.tile([P, 4, HD], BF16, name="fk_e", tag="fk_e")
                kp = fmk.tile([P, 4, HD], BF16, name="kp", tag="kp")
                nc.scalar.activation(t, ktile, AF.Relu, scale=-1.0)
                nc.scalar.activation(e, t, AF.Exp, scale=-1.0)
                nc.vector.scalar_tensor_tensor(
                    kp, in0=ktile, scalar=0.0, in1=e, op0=OP.max, op1=OP.add
                )
                first = cc == 0
                last = cc == n_kc - 1
                for sub in range(4):
                    for j in range(4):
                        lhsT = kp[:, sub, j * P : (j + 1) * P]
                        nc.tensor.matmul(
                            kv_ps[j],
                            lhsT=lhsT,
                            rhs=vtile[:, sub, j * P : (j + 1) * P],
                            start=(first and sub == 0),
                            stop=(last and sub == 3),
                        )
                        nc.tensor.matmul(
                            ks_ps[:, j : j + 1],
                            lhsT=lhsT,
                            rhs=ones,
                            start=(first and sub == 0),
                            stop=(last and sub == 3),
                        )

            # ---- Phase 1.5: block-diag KV weights (bf16) + Ksum blocks ----
            w2 = [wp.tile([P, P], BF16, name=f"w2_{j}", tag=f"w2_{j}") for j in range(4)]
            kb = [wp.tile([P, 8], BF16, name=f"kb_{j}", tag=f"kb_{j}") for j in range(4)]
            for j in range(4):
                nc.vector.memset(w2[j], 0.0)
                nc.vector.tensor_copy(w2[j][0:64, 0:64], kv_ps[j][0:64, 0:64])
                nc.vector.tensor_copy(w2[j][64:128, 64:128], kv_ps[j][64:128, 64:128])
                nc.vector.memset(kb[j], 0.0)
                nc.vector.tensor_copy(
                    kb[j][0:64, 2 * j : 2 * j + 1], ks_ps[0:64, j : j + 1]
                )
                nc.vector.tensor_copy(
                    kb[j][64:128, 2 * j + 1 : 2 * j + 2], ks_ps[64:128, j : j + 1]
                )

            # ---- Phase 2: stream Q ----
            n_sub = QC // P  # 16 sub-chunks of 128 l-rows per mega-chunk
            for qc in range(n_qc):
                l0 = qc * QC
                qp = []
                for g in range(4):
                    qt = qdma.tile([P, QC], BF16, name=f"qt{g}", tag=f"qt{g}")
                    nc.sync.dma_start_transpose(
                        qt, q_d[l0 : l0 + QC, g * P : (g + 1) * P]
                    )
                    t = fmq.tile([P, QC], BF16, name=f"fq_t{g}", tag=f"fq_t{g}")
                    e = fmq.tile([P, QC], BF16, name=f"fq_e{g}", tag=f"fq_e{g}")
                    qpg = fmq.tile([P, QC], BF16, name=f"qp{g}", tag=f"qp{g}")
                    nc.scalar.activation(t, qt, AF.Relu, scale=-1.0)
                    nc.scalar.activation(e, t, AF.Exp, scale=-1.0)
                    nc.vector.scalar_tensor_tensor(
                        qpg, in0=qt, scalar=0.0, in1=e, op0=OP.max, op1=OP.add
                    )
                    qp.append(qpg)

                for blk in range(QC // KC):  # 4 blocks of 512 l-rows
                    otile = outp.tile([P, 4, HD], FP32, name="otile", tag="otile")
                    for sub4 in range(4):
                        sub = blk * 4 + sub4
                        out_ps = opsp.tile([P, HD], FP32, name="op", tag="op")
                        den_ps = dpsp.tile([P, 8], FP32, name="den", tag="den")
                        for g in range(4):
                            lhsT = qp[g][:, sub * P : (sub + 1) * P]
                            nc.tensor.matmul(
                                out_ps[:, g * P : (g + 1) * P],
                                lhsT=lhsT,
                                rhs=w2[g],
                                start=True,
                                stop=True,
                            )
                            nc.tensor.matmul(
                                den_ps,
                                lhsT=lhsT,
                                rhs=kb[g],
                                start=(g == 0),
                                stop=(g == 3),
                            )
                        zr = zp.tile([P, 8], FP32, name="zr", tag="zr")
                        nc.vector.reciprocal(zr, den_ps)
                        zb = zr.unsqueeze(2).to_broadcast([P, 8, 64])
                        nc.vector.tensor_mul(
                            otile[:, sub4, :].rearrange("p (h v) -> p h v", h=H),
                            out_ps.rearrange("p (h v) -> p h v", h=H),
                            zb,
                        )
                    nc.sync.dma_start(
                        o_d[l0 + blk * KC : l0 + (blk + 1) * KC, :].rearrange(
                            "(c p) f -> p c f", p=P
                        ),
                        otile,
                    )
    nc.compile()
    return nc


@functools.lru_cache(maxsize=None)
def _cached_nc(L_, S_):
    return build_kernel(L_, S_)


def _to_bf16(x: np.ndarray) -> np.ndarray:
    """fp32 -> bf16 with round-to-nearest-even (vectorized, no ml_dtypes cast)."""
    u = np.ascontiguousarray(x, np.float32).view(np.uint32)
    r = (u + np.uint32(0x7FFF) + ((u >> np.uint32(16)) & np.uint32(1))) >> np.uint32(16)
    return r.astype(np.uint16).view(ml_dtypes.bfloat16)


def kernel(queries: np.ndarray, keys: np.ndarray, values: np.ndarray) -> np.ndarray:
    n, l_, h, d = queries.shape
    s_ = keys.shape[1]
    hd = h * d
    q = _to_bf16(np.asarray(queries, np.float32).reshape(n, l_, hd))
    k = _to_bf16(np.asarray(keys, np.float32).reshape(n, s_, hd))
    v = _to_bf16(np.asarray(values, np.float32).reshape(n, s_, hd))
    nc = _cached_nc(l_, s_)
    in_maps = [
        {"queries": q[i], "keys": k[i], "values": v[i]} for i in range(n)
    ]
    res = run_bass_kernel_spmd(nc, in_maps, core_ids=list(range(n)))
    out = np.empty((n, l_, h, d), np.float32)
    for i in range(n):
        out[i] = res.results[i]["out"].reshape(l_, h, d)
    return out


if __name__ == "__main__":
    nc = build_kernel()
    print("build ok")


# revision 11
# speedup vs baseline: 460.9595x; 1.0610x over previous
"""Linear attention (elu+1 feature map) Bass/Tile kernel for Trainium2.

Full inputs: queries/keys/values [N=8, L/S=8192, H=8, D=64] fp32.
Sharding: data-parallel over N across the 8 NeuronCores (batch i -> core i).
Inputs are cast to bf16 on the host (round-to-nearest) to halve DMA volume
and enable bf16 matmul / FWL / fast DVE modes; all PSUM accumulation is fp32.

Math per (n, h):
  Q' = elu(Q)+1, K' = elu(K)+1
  KV[d, v] = sum_s K'[s, d] V[s, v]     (the /S, *S in the reference cancel
  Ksum[d]  = sum_s K'[s, d]              exactly: S = 2^13)
  out[l, v] = (Q'[l, :] @ KV[:, v]) / (Q'[l, :] @ Ksum)   (EPS=1e-6 dropped:
  denom ~ O(5000), the reference's eps is far below bf16 noise)

Kernel structure per core:
  Phase 1 (stream K, V in 512-row chunks): feature-map K on ACT+DVE, then per
    head PAIR one matmul lhsT=K'_pair [128, 128], rhs=V_pair [128, 128]
    accumulated into psum KV_j [128, 128] (cross-head blocks are computed but
    discarded), plus a tiny N=1 matmul with a ones column accumulating
    Ksum into a shared [128, 4] psum bank.
  Phase 1.5: copy KV block-diagonals to bf16 weights w2[j] [128, 128]
    (cross blocks zeroed) and scatter Ksum into block rhs kb[g] [128, 8].
  Phase 2 (stream Q in 2048-row mega-chunks): per head-group g, xbar
    DMA-transpose loads Q^T [128 hd, 2048 l] directly from DRAM; feature-map;
    then per 128-l sub-chunk: 4 matmuls (lhsT=Q'^T slice, rhs=w2[g]) fill one
    [128, 512] out psum + 4 N=8 matmuls accumulate all-head denominators into
    a [128, 32] psum (4 subs batched); one reciprocal per 4 subs; epilogue is
    a single tensor_tensor multiply with a stride-0 broadcast of 1/denom.
"""

import functools
import sys

sys.path.insert(0, "/opt/trn_rl_repo")

import numpy as np
import ml_dtypes

import concourse.bass as bass
import concourse.mybir as mybir
import concourse.tile as tile
from concourse import bacc
from concourse.bass_utils import run_bass_kernel_spmd

N, L, S, H, D = 8, 8192, 8192, 8, 64
P = 128
HD = H * D
BF16 = mybir.dt.bfloat16
FP32 = mybir.dt.float32
AF = mybir.ActivationFunctionType
OP = mybir.AluOpType
KC = 512    # K/V chunk rows
QC = 2048   # Q mega-chunk rows


def build_kernel(L_=L, S_=S):
    nc = bacc.Bacc(trn_type="TRN2")
    q_d = nc.dram_tensor("queries", [L_, HD], BF16, kind="ExternalInput")
    k_d = nc.dram_tensor("keys", [S_, HD], BF16, kind="ExternalInput")
    v_d = nc.dram_tensor("values", [S_, HD], BF16, kind="ExternalInput")
    o_d = nc.dram_tensor("out", [L_, HD], FP32, kind="ExternalOutput")

    n_kc = S_ // KC
    n_qc = L_ // QC

    with tile.TileContext(nc) as tc:
        with (
            tc.tile_pool(name="consts", bufs=1) as consts,
            tc.tile_pool(name="kdma", bufs=3) as kdma,
            tc.tile_pool(name="vdma", bufs=3) as vdma,
            tc.tile_pool(name="fmk", bufs=2) as fmk,
            tc.tile_pool(name="wp", bufs=1) as wp,
            tc.tile_pool(name="qdma", bufs=4) as qdma,
            tc.tile_pool(name="fmq", bufs=2) as fmq,
            tc.tile_pool(name="zp", bufs=2) as zp,
            tc.tile_pool(name="outp", bufs=3) as outp,
            tc.tile_pool(name="kvps", bufs=1, space="PSUM") as kvps,
            tc.tile_pool(name="ops", bufs=2, space="PSUM") as opsp,
            tc.tile_pool(name="dps", bufs=2, space="PSUM") as dpsp,
        ):
            ones = consts.tile([P, 1], BF16)
            nc.vector.memset(ones, 1.0)

            # ---- Phase 1: KV + Ksum accumulation ----
            # Per head pair j one psum bank: cols 0:128 = KV outer products
            # (cross-head blocks discarded later), col 128 = Ksum via a ones
            # column.  The Ksum chain never uses start=True: the V-chain's
            # first matmul clears the bank's has_written bits, so the first
            # ones-matmul overwrites col 128 per-element and accumulates after.
            kv_ps = [
                kvps.tile([P, 129], FP32, name=f"kv{j}", tag=f"kv{j}") for j in range(4)
            ]

            for cc in range(n_kc):
                r0 = cc * KC
                ktile = kdma.tile([P, 4, HD], BF16, name="ktile", tag="ktile")
                nc.sync.dma_start(
                    ktile, k_d[r0 : r0 + KC, :].rearrange("(c p) f -> p c f", p=P)
                )
                vtile = vdma.tile([P, 4, HD], BF16, name="vtile", tag="vtile")
                nc.sync.dma_start(
                    vtile, v_d[r0 : r0 + KC, :].rearrange("(c p) f -> p c f", p=P)
                )
                # kp = elu(ktile)+1 = max(x,0) + exp(-relu(-x)).  t/e stay
                # fp32: the ACT Exp table in bf16 is ~3.7e-3 inaccurate
                # (measured) vs 1e-5 in fp32; only kp's final store is bf16.
                t = fmk.tile([P, 4, HD], FP32, name="fk_t", tag="fk_t")
                e = fmk.tile([P, 4, HD], FP32, name="fk_e", tag="fk_e")
                kp = fmk.tile([P, 4, HD], BF16, name="kp", tag="kp")
                nc.scalar.activation(t, ktile, AF.Relu, scale=-1.0)
                nc.scalar.activation(e, t, AF.Exp, scale=-1.0)
                nc.vector.scalar_tensor_tensor(
                    kp, in0=ktile, scalar=0.0, in1=e, op0=OP.max, op1=OP.add
                )
                first = cc == 0
                last = cc == n_kc - 1
                for sub in range(4):
                    for j in range(4):
                        lhsT = kp[:, sub, j * P : (j + 1) * P]
                        nc.tensor.matmul(
                            kv_ps[j][:, 0:P],
                            lhsT=lhsT,
                            rhs=vtile[:, sub, j * P : (j + 1) * P],
                            start=(first and sub == 0),
                            stop=(last and sub == 3),
                        )
                        nc.tensor.matmul(
                            kv_ps[j][:, P : P + 1],
                            lhsT=lhsT,
                            rhs=ones,
                            start=False,
                            stop=False,
                            skip_group_check=True,
                        )

            # ---- Phase 1.5: block-diag KV weights (bf16) + Ksum blocks ----
            w2 = [wp.tile([P, P], BF16, name=f"w2_{j}", tag=f"w2_{j}") for j in range(4)]
            kb = [wp.tile([P, 8], BF16, name=f"kb_{j}", tag=f"kb_{j}") for j in range(4)]
            for j in range(4):
                nc.vector.memset(w2[j], 0.0)
                nc.vector.tensor_copy(w2[j][0:64, 0:64], kv_ps[j][0:64, 0:64])
                nc.vector.tensor_copy(w2[j][64:128, 64:128], kv_ps[j][64:128, 64:128])
                nc.vector.memset(kb[j], 0.0)
                nc.vector.tensor_copy(
                    kb[j][0:64, 2 * j : 2 * j + 1], kv_ps[j][0:64, P : P + 1]
                )
                nc.vector.tensor_copy(
                    kb[j][64:128, 2 * j + 1 : 2 * j + 2], kv_ps[j][64:128, P : P + 1]
                )

            # ---- Phase 2: stream Q ----
            n_sub = QC // P  # 16 sub-chunks of 128 l-rows per mega-chunk
            for qc in range(n_qc):
                l0 = qc * QC
                qp = []
                for g in range(4):
                    qt = qdma.tile([P, QC], BF16, name=f"qt{g}", tag="qt")
                    nc.sync.dma_start_transpose(
                        qt, q_d[l0 : l0 + QC, g * P : (g + 1) * P]
                    )
                    t = fmq.tile([P, QC], FP32, name=f"fq_t{g}", tag="fq_t")
                    e = fmq.tile([P, QC], FP32, name=f"fq_e{g}", tag="fq_e")
                    qpg = fmq.tile([P, QC], BF16, name=f"qp{g}", tag=f"qp{g}")
                    nc.scalar.activation(t, qt, AF.Relu, scale=-1.0)
                    nc.scalar.activation(e, t, AF.Exp, scale=-1.0)
                    nc.vector.scalar_tensor_tensor(
                        qpg, in0=qt, scalar=0.0, in1=e, op0=OP.max, op1=OP.add
                    )
                    qp.append(qpg)

                for blk in range(QC // KC):  # 4 blocks of 512 l-rows
                    otile = outp.tile([P, 4, HD], FP32, name="otile", tag="otile")
                    for sub4 in range(4):
                        sub = blk * 4 + sub4
                        out_ps = opsp.tile([P, HD], FP32, name="op", tag="op")
                        den_ps = dpsp.tile([P, 8], FP32, name="den", tag="den")
                        # denominator matmuls first so the reciprocal can
                        # overlap the output matmuls
                        for g in range(4):
                            nc.tensor.matmul(
                                den_ps,
                                lhsT=qp[g][:, sub * P : (sub + 1) * P],
                                rhs=kb[g],
                                start=(g == 0),
                                stop=(g == 3),
                            )
                        for g in range(4):
                            nc.tensor.matmul(
                                out_ps[:, g * P : (g + 1) * P],
                                lhsT=qp[g][:, sub * P : (sub + 1) * P],
                                rhs=w2[g],
                                start=True,
                                stop=True,
                            )
                        zr = zp.tile([P, 8], FP32, name="zr", tag="zr")
                        nc.vector.reciprocal(zr, den_ps)
                        zb = zr.unsqueeze(2).to_broadcast([P, 8, 64])
                        nc.vector.tensor_mul(
                            otile[:, sub4, :].rearrange("p (h v) -> p h v", h=H),
                            out_ps.rearrange("p (h v) -> p h v", h=H),
                            zb,
                        )
                    nc.sync.dma_start(
                        o_d[l0 + blk * KC : l0 + (blk + 1) * KC, :].rearrange(
                            "(c p) f -> p c f", p=P
                        ),
                        otile,
                    )
    nc.compile()
    return nc


@functools.lru_cache(maxsize=None)
def _cached_nc(L_, S_):
    return build_kernel(L_, S_)


def _to_bf16(x: np.ndarray) -> np.ndarray:
    """fp32 -> bf16 with round-to-nearest-even (vectorized, no ml_dtypes cast)."""
    u = np.ascontiguousarray(x, np.float32).view(np.uint32)
    r = (u + np.uint32(0x7FFF) + ((u >> np.uint32(16)) & np.uint32(1))) >> np.uint32(16)
    return r.astype(np.uint16).view(ml_dtypes.bfloat16)


def kernel(queries: np.ndarray, keys: np.ndarray, values: np.ndarray) -> np.ndarray:
    n, l_, h, d = queries.shape
    s_ = keys.shape[1]
    hd = h * d
    q = _to_bf16(np.asarray(queries, np.float32).reshape(n, l_, hd))
    k = _to_bf16(np.asarray(keys, np.float32).reshape(n, s_, hd))
    v = _to_bf16(np.asarray(values, np.float32).reshape(n, s_, hd))
    nc = _cached_nc(l_, s_)
    in_maps = [
        {"queries": q[i], "keys": k[i], "values": v[i]} for i in range(n)
    ]
    res = run_bass_kernel_spmd(nc, in_maps, core_ids=list(range(n)))
    out = np.empty((n, l_, h, d), np.float32)
    for i in range(n):
        out[i] = res.results[i]["out"].reshape(l_, h, d)
    return out


if __name__ == "__main__":
    nc = build_kernel()
    print("build ok")


# revision 12
# speedup vs baseline: 474.7007x; 1.0298x over previous
"""Linear attention (elu+1 feature map) Bass/Tile kernel for Trainium2.

Full inputs: queries/keys/values [N=8, L/S=8192, H=8, D=64] fp32.
Sharding: data-parallel over N across the 8 NeuronCores (batch i -> core i).
Inputs are cast to bf16 on the host (round-to-nearest); output is bf16 and
upcast on the host.  All PSUM accumulation is fp32.

Math per (n, h):
  Q' = elu(Q)+1, K' = elu(K)+1
  KV[d, v] = sum_s K'[s, d] V[s, v]     (the /S, *S in the reference cancel
  Ksum[d]  = sum_s K'[s, d]              exactly: S = 2^13)
  out[l, v] = (Q'[l, :] @ KV[:, v]) / (Q'[l, :] @ Ksum)   (EPS dropped:
  denom ~ O(5000), the reference's 1e-6 is far below bf16 noise)

Feature map fm(x) = max(x,0) + exp(min(x,0)), two engine-balanced variants:
  chain C: m = min(x,0) [DVE 4x, exact]; e = Exp(m) -> fp32 [ACT, table is
    only accurate with fp32 out]; fm = STT max(x,0)+e [DVE 1x]
  chain G: m, r = min/max(x,0) [DVE 4x]; e = Exp(m) -> bf16 [ACT, ~2ulp];
    fm = r + e on GPSIMD [otherwise idle engine]
Tiles alternate variants to balance DVE vs GPSIMD occupancy.

Phase 1 (stream K, V in 512-row chunks): per head PAIR one matmul
  lhsT=K'_pair [128,128] rhs=V_pair accumulated into psum [128, 0:128], plus
  an N=1 ones-matmul into col 128 of the same bank for Ksum (start=False
  always: the V-chain's start clears the bank's has_written bits, so the
  first ones-matmul overwrites per-element and accumulates after).
Phase 1.5: copy KV block-diagonals to bf16 w2[j] [128,128] (cross blocks
  zeroed), scatter Ksum into block rhs kb[g] [128, 8].
Phase 2 (stream Q in 2048-row mega-chunks): per head-group g an xbar
  DMA-transpose loads Q^T [128 hd, 2048 l] straight from DRAM (measured
  bit-exact); feature-map; then per 512-l block: 16 N=8 denominator matmuls
  into one [128, 32] psum, one reciprocal, and per 128-l sub 4 output
  matmuls into a [128, 512] psum followed by a single broadcast
  tensor_tensor multiply into the bf16 out tile.
"""

import functools
import sys

sys.path.insert(0, "/opt/trn_rl_repo")

import numpy as np
import ml_dtypes

import concourse.bass as bass
import concourse.mybir as mybir
import concourse.tile as tile
from concourse import bacc
from concourse.bass_utils import run_bass_kernel_spmd

N, L, S, H, D = 8, 8192, 8192, 8, 64
P = 128
HD = H * D
BF16 = mybir.dt.bfloat16
FP32 = mybir.dt.float32
AF = mybir.ActivationFunctionType
OP = mybir.AluOpType
KC = 512    # K/V chunk rows
QC = 2048   # Q mega-chunk rows


def _fm(nc, pool, x, out, shape, tag, gpsimd):
    """out = elu(x)+1 = max(x,0) + exp(min(x,0)), bf16 in/out."""
    m = pool.tile(shape, BF16, name=f"fm_m_{tag}", tag=f"fm_m_{tag[0]}")
    nc.vector.tensor_scalar_min(m, x, 0.0)
    if gpsimd:
        e = pool.tile(shape, BF16, name=f"fm_eb_{tag}", tag=f"fm_eb_{tag[0]}")
        nc.scalar.activation(e, m, AF.Exp)
        r = pool.tile(shape, BF16, name=f"fm_r_{tag}", tag=f"fm_r_{tag[0]}")
        nc.vector.tensor_scalar_max(r, x, 0.0)
        nc.gpsimd.tensor_add(out, r, e)
    else:
        e = pool.tile(shape, FP32, name=f"fm_ef_{tag}", tag=f"fm_ef_{tag[0]}")
        nc.scalar.activation(e, m, AF.Exp)
        nc.vector.scalar_tensor_tensor(
            out, in0=x, scalar=0.0, in1=e, op0=OP.max, op1=OP.add
        )


def build_kernel(L_=L, S_=S):
    nc = bacc.Bacc(trn_type="TRN2")
    q_d = nc.dram_tensor("queries", [L_, HD], BF16, kind="ExternalInput")
    k_d = nc.dram_tensor("keys", [S_, HD], BF16, kind="ExternalInput")
    v_d = nc.dram_tensor("values", [S_, HD], BF16, kind="ExternalInput")
    o_d = nc.dram_tensor("out", [L_, HD], BF16, kind="ExternalOutput")

    n_kc = S_ // KC
    n_qc = L_ // QC

    with tile.TileContext(nc) as tc:
        with (
            tc.tile_pool(name="consts", bufs=1) as consts,
            tc.tile_pool(name="kdma", bufs=3) as kdma,
            tc.tile_pool(name="vdma", bufs=3) as vdma,
            tc.tile_pool(name="fmk", bufs=2) as fmk,
            tc.tile_pool(name="wp", bufs=1) as wp,
            tc.tile_pool(name="qdma", bufs=4) as qdma,
            tc.tile_pool(name="fmq", bufs=2) as fmq,
            tc.tile_pool(name="zp", bufs=2) as zp,
            tc.tile_pool(name="outp", bufs=3) as outp,
        ):
            ones = consts.tile([P, 1], BF16)
            nc.vector.memset(ones, 1.0)

            w2 = [wp.tile([P, P], BF16, name=f"w2_{j}", tag=f"w2_{j}") for j in range(4)]
            kb = [wp.tile([P, 8], BF16, name=f"kb_{j}", tag=f"kb_{j}") for j in range(4)]

            # ---- Phase 1: KV + Ksum accumulation ----
            with tc.tile_pool(name="kvps", bufs=1, space="PSUM") as kvps:
                kv_ps = [
                    kvps.tile([P, 129], FP32, name=f"kv{j}", tag=f"kv{j}")
                    for j in range(4)
                ]
                for cc in range(n_kc):
                    r0 = cc * KC
                    ktile = kdma.tile([P, 4, HD], BF16, name="ktile", tag="ktile")
                    nc.sync.dma_start(
                        ktile, k_d[r0 : r0 + KC, :].rearrange("(c p) f -> p c f", p=P)
                    )
                    vtile = vdma.tile([P, 4, HD], BF16, name="vtile", tag="vtile")
                    nc.sync.dma_start(
                        vtile, v_d[r0 : r0 + KC, :].rearrange("(c p) f -> p c f", p=P)
                    )
                    kp = fmk.tile([P, 4, HD], BF16, name="kp", tag="kp")
                    _fm(nc, fmk, ktile, kp, [P, 4, HD], f"k{cc}", gpsimd=(cc % 2 == 0))
                    first = cc == 0
                    last = cc == n_kc - 1
                    for sub in range(4):
                        for j in range(4):
                            lhsT = kp[:, sub, j * P : (j + 1) * P]
                            nc.tensor.matmul(
                                kv_ps[j][:, 0:P],
                                lhsT=lhsT,
                                rhs=vtile[:, sub, j * P : (j + 1) * P],
                                start=(first and sub == 0),
                                stop=(last and sub == 3),
                            )
                            nc.tensor.matmul(
                                kv_ps[j][:, P : P + 1],
                                lhsT=lhsT,
                                rhs=ones,
                                start=False,
                                stop=False,
                                skip_group_check=True,
                            )

                # ---- Phase 1.5: block-diag KV weights (bf16) + Ksum blocks ----
                for j in range(4):
                    nc.vector.memset(w2[j], 0.0)
                    nc.vector.tensor_copy(w2[j][0:64, 0:64], kv_ps[j][0:64, 0:64])
                    nc.vector.tensor_copy(
                        w2[j][64:128, 64:128], kv_ps[j][64:128, 64:128]
                    )
                    nc.vector.memset(kb[j], 0.0)
                    nc.vector.tensor_copy(
                        kb[j][0:64, 2 * j : 2 * j + 1], kv_ps[j][0:64, P : P + 1]
                    )
                    nc.vector.tensor_copy(
                        kb[j][64:128, 2 * j + 1 : 2 * j + 2],
                        kv_ps[j][64:128, P : P + 1],
                    )

            # ---- Phase 2: stream Q ----
            with (
                tc.tile_pool(name="ops", bufs=4, space="PSUM") as opsp,
                tc.tile_pool(name="dps", bufs=2, space="PSUM") as dpsp,
            ):
                for qc in range(n_qc):
                    l0 = qc * QC
                    qp = []
                    for g in range(4):
                        qt = qdma.tile([P, QC], BF16, name=f"qt{g}", tag="qt")
                        nc.sync.dma_start_transpose(
                            qt, q_d[l0 : l0 + QC, g * P : (g + 1) * P]
                        )
                        qpg = fmq.tile([P, QC], BF16, name=f"qp{g}", tag=f"qp{g}")
                        _fm(
                            nc, fmq, qt, qpg, [P, QC], f"q{qc}_{g}",
                            gpsimd=((qc * 4 + g) % 4 != 3),
                        )
                        qp.append(qpg)

                    for blk in range(QC // KC):  # 4 blocks of 512 l-rows
                        den_ps = dpsp.tile([P, 32], FP32, name="den", tag="den")
                        for sub4 in range(4):
                            sub = blk * 4 + sub4
                            for g in range(4):
                                nc.tensor.matmul(
                                    den_ps[:, sub4 * 8 : (sub4 + 1) * 8],
                                    lhsT=qp[g][:, sub * P : (sub + 1) * P],
                                    rhs=kb[g],
                                    start=(g == 0),
                                    stop=(g == 3),
                                    skip_group_check=True,
                                )
                        zr = zp.tile([P, 32], FP32, name="zr", tag="zr")
                        nc.vector.reciprocal(zr, den_ps)
                        otile = outp.tile([P, 4, HD], BF16, name="otile", tag="otile")
                        for sub4 in range(4):
                            sub = blk * 4 + sub4
                            out_ps = opsp.tile([P, HD], FP32, name="op", tag="op")
                            for g in range(4):
                                nc.tensor.matmul(
                                    out_ps[:, g * P : (g + 1) * P],
                                    lhsT=qp[g][:, sub * P : (sub + 1) * P],
                                    rhs=w2[g],
                                    start=True,
                                    stop=True,
                                )
                            zb = (
                                zr[:, sub4 * 8 : (sub4 + 1) * 8]
                                .unsqueeze(2)
                                .to_broadcast([P, 8, 64])
                            )
                            nc.vector.tensor_mul(
                                otile[:, sub4, :].rearrange("p (h v) -> p h v", h=H),
                                out_ps.rearrange("p (h v) -> p h v", h=H),
                                zb,
                            )
                        nc.sync.dma_start(
                            o_d[l0 + blk * KC : l0 + (blk + 1) * KC, :].rearrange(
                                "(c p) f -> p c f", p=P
                            ),
                            otile,
                        )
    nc.compile()
    return nc


@functools.lru_cache(maxsize=None)
def _cached_nc(L_, S_):
    return build_kernel(L_, S_)


def _to_bf16(x: np.ndarray) -> np.ndarray:
    """fp32 -> bf16 with round-to-nearest-even (vectorized, no ml_dtypes cast)."""
    u = np.ascontiguousarray(x, np.float32).view(np.uint32)
    r = (u + np.uint32(0x7FFF) + ((u >> np.uint32(16)) & np.uint32(1))) >> np.uint32(16)
    return r.astype(np.uint16).view(ml_dtypes.bfloat16)


def _from_bf16(x: np.ndarray) -> np.ndarray:
    u = np.ascontiguousarray(x).view(np.uint16).astype(np.uint32) << np.uint32(16)
    return u.view(np.float32)


def kernel(queries: np.ndarray, keys: np.ndarray, values: np.ndarray) -> np.ndarray:
    n, l_, h, d = queries.shape
    s_ = keys.shape[1]
    hd = h * d
    q = _to_bf16(np.asarray(queries, np.float32).reshape(n, l_, hd))
    k = _to_bf16(np.asarray(keys, np.float32).reshape(n, s_, hd))
    v = _to_bf16(np.asarray(values, np.float32).reshape(n, s_, hd))
    nc = _cached_nc(l_, s_)
    in_maps = [{"queries": q[i], "keys": k[i], "values": v[i]} for i in range(n)]
    res = run_bass_kernel_spmd(nc, in_maps, core_ids=list(range(n)))
    out = np.empty((n, l_, h, d), np.float32)
    for i in range(n):
        out[i] = _from_bf16(res.results[i]["out"]).reshape(l_, h, d)
    return out


if __name__ == "__main__":
    nc = build_kernel()
    print("build ok")


# revision 18
# speedup vs baseline: 494.9111x; 1.0426x over previous
"""Linear attention (elu+1 feature map) Bass/Tile kernel for Trainium2.

Full inputs: queries/keys/values [N=8, L/S=8192, H=8, D=64] fp32.
Sharding: data-parallel over N across the 8 NeuronCores (batch i -> core i).
Inputs are cast to bf16 on the host (round-to-nearest); output is bf16 and
upcast on the host.  All PSUM accumulation is fp32.

Math per (n, h):
  Q' = elu(Q)+1, K' = elu(K)+1
  KV[d, v] = sum_s K'[s, d] V[s, v]     (the /S, *S in the reference cancel
  Ksum[d]  = sum_s K'[s, d]              exactly: S = 2^13)
  out[l, v] = (Q'[l, :] @ KV[:, v]) / (Q'[l, :] @ Ksum)   (EPS dropped:
  denom ~ O(5000), the reference's 1e-6 is far below bf16 noise)

Feature map fm(x) = max(x,0) + exp(min(x,0)), two engine-balanced variants:
  chain C: m = min(x,0) [DVE 4x, exact]; e = Exp(m) -> fp32 [ACT, table is
    only accurate with fp32 out]; fm = STT max(x,0)+e [DVE 1x]
  chain G: m, r = min/max(x,0) [DVE 4x]; e = Exp(m) -> bf16 [ACT, ~2ulp];
    fm = r + e on GPSIMD [otherwise idle engine]
Tiles alternate variants to balance DVE vs GPSIMD occupancy.

Phase 1 (stream K, V in 512-row chunks): per head PAIR one matmul
  lhsT=K'_pair [128,128] rhs=V_pair accumulated into psum [128, 0:128], plus
  an N=1 ones-matmul into col 128 of the same bank for Ksum (start=False
  always: the V-chain's start clears the bank's has_written bits, so the
  first ones-matmul overwrites per-element and accumulates after).
Phase 1.5: copy KV block-diagonals to bf16 w2[j] [128,128] (cross blocks
  zeroed), scatter Ksum into block rhs kb[g] [128, 8].
Phase 2 (stream Q in 2048-row mega-chunks): per head-group g an xbar
  DMA-transpose loads Q^T [128 hd, 2048 l] straight from DRAM (measured
  bit-exact); feature-map; then per 512-l block: 16 N=8 denominator matmuls
  into one [128, 32] psum, one reciprocal, and per 128-l sub 4 output
  matmuls into a [128, 512] psum followed by a single broadcast
  tensor_tensor multiply into the bf16 out tile.
"""

import functools
import sys

sys.path.insert(0, "/opt/trn_rl_repo")

import numpy as np
import ml_dtypes

import concourse.bass as bass
import concourse.mybir as mybir
import concourse.tile as tile
from concourse import bacc
from concourse.bass_utils import run_bass_kernel_spmd

N, L, S, H, D = 8, 8192, 8192, 8, 64
P = 128
HD = H * D
BF16 = mybir.dt.bfloat16
FP32 = mybir.dt.float32
AF = mybir.ActivationFunctionType
OP = mybir.AluOpType
KC = 512    # K/V chunk rows
QC = 2048   # Q mega-chunk rows


def _fm(nc, pool, x, out, shape, tag, gpsimd):
    """out = elu(x)+1 = max(x,0) + exp(min(x,0)), bf16 in/out."""
    m = pool.tile(shape, BF16, name=f"fm_m_{tag}", tag=f"fm_m_{tag[0]}")
    nc.vector.tensor_scalar_min(m, x, 0.0)
    if gpsimd:
        e = pool.tile(shape, BF16, name=f"fm_eb_{tag}", tag=f"fm_eb_{tag[0]}")
        nc.scalar.activation(e, m, AF.Exp)
        r = pool.tile(shape, BF16, name=f"fm_r_{tag}", tag=f"fm_r_{tag[0]}")
        nc.vector.tensor_scalar_max(r, x, 0.0)
        nc.gpsimd.tensor_add(out, r, e)
    else:
        e = pool.tile(shape, FP32, name=f"fm_ef_{tag}", tag=f"fm_ef_{tag[0]}")
        nc.scalar.activation(e, m, AF.Exp)
        nc.vector.scalar_tensor_tensor(
            out, in0=x, scalar=0.0, in1=e, op0=OP.max, op1=OP.add
        )


def build_kernel(L_=L, S_=S):
    nc = bacc.Bacc(trn_type="TRN2")
    q_d = nc.dram_tensor("queries", [L_, HD], BF16, kind="ExternalInput")
    k_d = nc.dram_tensor("keys", [S_, HD], BF16, kind="ExternalInput")
    # values are host-packed [S, 4 pairs, 130]: cols 0:128 = the pair's V
    # columns, col 128 = 1.0 (folds the Ksum ones-column into the KV matmul),
    # col 129 = alignment pad
    v_d = nc.dram_tensor("values", [S_, 4 * 130], BF16, kind="ExternalInput")
    o_d = nc.dram_tensor("out", [L_, HD], BF16, kind="ExternalOutput")

    n_kc = S_ // KC
    n_qc = L_ // QC

    with tile.TileContext(nc) as tc:
        with (
            tc.tile_pool(name="kdma", bufs=3) as kdma,
            tc.tile_pool(name="vdma", bufs=3) as vdma,
            tc.tile_pool(name="fmk", bufs=2) as fmk,
            tc.tile_pool(name="wp", bufs=1) as wp,
            tc.tile_pool(name="qdma", bufs=4) as qdma,
            tc.tile_pool(name="fmq", bufs=2) as fmq,
            tc.tile_pool(name="zp", bufs=2) as zp,
            tc.tile_pool(name="outp", bufs=3) as outp,
        ):
            w2 = [wp.tile([P, P], BF16, name=f"w2_{j}", tag=f"w2_{j}") for j in range(4)]
            kb = [wp.tile([P, 8], BF16, name=f"kb_{j}", tag=f"kb_{j}") for j in range(4)]

            # ---- Phase 1: KV + Ksum accumulation ----
            with tc.tile_pool(name="kvps", bufs=1, space="PSUM") as kvps:
                kv_ps = [
                    kvps.tile([P, 129], FP32, name=f"kv{j}", tag=f"kv{j}")
                    for j in range(4)
                ]
                for cc in range(n_kc):
                    r0 = cc * KC
                    ktile = kdma.tile([P, 4 * HD], BF16, name="ktile", tag="ktile")
                    nc.sync.dma_start(
                        ktile.rearrange("p (c f) -> p c f", c=4),
                        k_d[r0 : r0 + KC, :].rearrange("(c p) f -> p c f", p=P),
                    )
                    vtile = vdma.tile([P, 4, 4, 130], BF16, name="vtile", tag="vtile")
                    nc.sync.dma_start(
                        vtile,
                        v_d[r0 : r0 + KC, :].rearrange(
                            "(c p) (j e) -> p c j e", p=P, j=4
                        ),
                    )
                    kp = fmk.tile([P, 4 * HD], BF16, name="kp", tag="kp")
                    _fm(nc, fmk, ktile, kp, [P, 4 * HD], f"k{cc}", gpsimd=(cc % 2 == 0))
                    first = cc == 0
                    last = cc == n_kc - 1
                    for sub in range(4):
                        for j in range(4):
                            nc.tensor.matmul(
                                kv_ps[j],
                                lhsT=kp[:, (sub * 4 + j) * P : (sub * 4 + j + 1) * P],
                                rhs=vtile[:, sub, j, 0:129],
                                start=(first and sub == 0),
                                stop=(last and sub == 3),
                            )

                # ---- Phase 1.5: block-diag KV weights (bf16) + Ksum blocks ----
                for j in range(4):
                    nc.vector.memset(w2[j], 0.0)
                    nc.vector.tensor_copy(w2[j][0:64, 0:64], kv_ps[j][0:64, 0:64])
                    nc.vector.tensor_copy(
                        w2[j][64:128, 64:128], kv_ps[j][64:128, 64:128]
                    )
                    nc.vector.memset(kb[j], 0.0)
                    nc.vector.tensor_copy(
                        kb[j][0:64, 2 * j : 2 * j + 1], kv_ps[j][0:64, P : P + 1]
                    )
                    nc.vector.tensor_copy(
                        kb[j][64:128, 2 * j + 1 : 2 * j + 2],
                        kv_ps[j][64:128, P : P + 1],
                    )

            # ---- Phase 2: stream Q ----
            with (
                tc.tile_pool(name="ops", bufs=4, space="PSUM") as opsp,
                tc.tile_pool(name="dps", bufs=2, space="PSUM") as dpsp,
            ):
                for qc in range(n_qc):
                    l0 = qc * QC
                    qp = []
                    for g in range(4):
                        qt = qdma.tile([P, QC], BF16, name=f"qt{g}", tag="qt")
                        nc.sync.dma_start_transpose(
                            qt, q_d[l0 : l0 + QC, g * P : (g + 1) * P]
                        )
                        qpg = fmq.tile([P, QC], BF16, name=f"qp{g}", tag=f"qp{g}")
                        _fm(
                            nc, fmq, qt, qpg, [P, QC], f"q{qc}_{g}",
                            gpsimd=((qc * 4 + g) % 4 != 3),
                        )
                        qp.append(qpg)

                    for blk in range(QC // KC):  # 4 blocks of 512 l-rows
                        den_ps = dpsp.tile([P, 32], FP32, name="den", tag="den")
                        for sub4 in range(4):
                            sub = blk * 4 + sub4
                            for g in range(4):
                                nc.tensor.matmul(
                                    den_ps[:, sub4 * 8 : (sub4 + 1) * 8],
                                    lhsT=qp[g][:, sub * P : (sub + 1) * P],
                                    rhs=kb[g],
                                    start=(g == 0),
                                    stop=(g == 3),
                                    skip_group_check=True,
                                )
                        zr = zp.tile([P, 32], FP32, name="zr", tag="zr")
                        nc.vector.reciprocal(zr, den_ps)
                        otile = outp.tile([P, 4, HD], BF16, name="otile", tag="otile")
                        for sub4 in range(4):
                            sub = blk * 4 + sub4
                            out_ps = opsp.tile([P, HD], FP32, name="op", tag="op")
                            for g in range(4):
                                nc.tensor.matmul(
                                    out_ps[:, g * P : (g + 1) * P],
                                    lhsT=qp[g][:, sub * P : (sub + 1) * P],
                                    rhs=w2[g],
                                    start=True,
                                    stop=True,
                                )
                            zb = (
                                zr[:, sub4 * 8 : (sub4 + 1) * 8]
                                .unsqueeze(2)
                                .to_broadcast([P, 8, 64])
                            )
                            nc.vector.tensor_mul(
                                otile[:, sub4, :].rearrange("p (h v) -> p h v", h=H),
                                out_ps.rearrange("p (h v) -> p h v", h=H),
                                zb,
                            )
                        nc.sync.dma_start(
                            o_d[l0 + blk * KC : l0 + (blk + 1) * KC, :].rearrange(
                                "(c p) f -> p c f", p=P
                            ),
                            otile,
                        )
    nc.compile()
    return nc


@functools.lru_cache(maxsize=None)
def _cached_nc(L_, S_):
    return build_kernel(L_, S_)


def _to_bf16(x: np.ndarray) -> np.ndarray:
    """fp32 -> bf16 with round-to-nearest-even (vectorized, no ml_dtypes cast)."""
    u = np.ascontiguousarray(x, np.float32).view(np.uint32)
    r = (u + np.uint32(0x7FFF) + ((u >> np.uint32(16)) & np.uint32(1))) >> np.uint32(16)
    return r.astype(np.uint16).view(ml_dtypes.bfloat16)


def _from_bf16(x: np.ndarray) -> np.ndarray:
    u = np.ascontiguousarray(x).view(np.uint16).astype(np.uint32) << np.uint32(16)
    return u.view(np.float32)


def _pack_values(values: np.ndarray) -> np.ndarray:
    """[N, S, HD] fp32 -> bf16 [N, S, 4*130] with a ones column per head pair."""
    n, s_, hd = values.shape
    v = _to_bf16(values).view(np.uint16)
    out = np.zeros((n, s_, 4, 130), np.uint16)
    out[..., 128] = np.uint16(0x3F80)  # 1.0 in bf16
    out[..., 0:128] = v.reshape(n, s_, 4, 128)
    return out.reshape(n, s_, 4 * 130).view(ml_dtypes.bfloat16)


def kernel(queries: np.ndarray, keys: np.ndarray, values: np.ndarray) -> np.ndarray:
    n, l_, h, d = queries.shape
    s_ = keys.shape[1]
    hd = h * d
    q = _to_bf16(np.asarray(queries, np.float32).reshape(n, l_, hd))
    k = _to_bf16(np.asarray(keys, np.float32).reshape(n, s_, hd))
    v = _pack_values(np.asarray(values, np.float32).reshape(n, s_, hd))
    nc = _cached_nc(l_, s_)
    in_maps = [{"queries": q[i], "keys": k[i], "values": v[i]} for i in range(n)]
    res = run_bass_kernel_spmd(nc, in_maps, core_ids=list(range(n)))
    out = np.empty((n, l_, h, d), np.float32)
    for i in range(n):
        out[i] = _from_bf16(res.results[i]["out"]).reshape(l_, h, d)
    return out


if __name__ == "__main__":
    nc = build_kernel()
    print("build ok")
